# revision 23
# baseline (speedup 1.0000x reference)
"""GCN graph-embedding kernel for 8 Trainium2 NeuronCores (Bass/Tile).

Strategy (dst-node sharding per the spec sharding_hint):
  - Nodes are permuted into 128-node blocks balanced by in-degree, 49
    positions per core. Per-position tile counts K[pos] are uniform across
    cores (SPMD, one program).
  - Aggregation runs on the TensorEngine: per 128-edge tile, a matmul with
    a host-precomputed bf16 selection matrix sel[e, dstslot] = norm_e.
    Self-loops use a diagonal selection scaled by 1/deg.
  - Layer 1's source rows are a pure permutation of the INPUT x, so its
    edge stream is pre-gathered host-side and streamed with cheap affine
    DMAs (no gpsimd work). Layer 2 must gather device-computed h rows with
    per-tile indirect DMAs on GpSimd (~1.1us each) - the critical resource.
  - To hide that cost, h is exchanged in 7 position-chunks: each chunk's
    AllGather (Shared-scratchpad output, HBM-speed) fires mid-layer-1 and
    is merge-copied into a chunk-major Local table h_local. Layer-2 edges
    are sorted per block by source chunk, so tile t only needs rows
    h_local[0:reqrows[t]]; gathers start ~60us into layer 1 and overlap it
    almost completely. Layer-2 tiles are processed wave-major with SBUF
    accumulators (PSUM bank count would not allow 49 open accumulations).
  - Global mean-pool is fused into layer 2's epilogues as one-hot matmuls
    accumulated in PSUM; partial graph sums are combined with a small
    AllReduce and every core finishes the tiny linear head redundantly.

The walrus build in this container rejects instructions with more than one
semaphore wait; split_multi_waits() rewrites the scheduled program so each
instruction carries at most one (extra waits move to same-engine NoOps).
"""
import numpy as np

import concourse.bass as bass
import concourse.mybir as mybir
import concourse.tile as tile
from concourse.bass_utils import run_bass_kernel_spmd
from concourse.tile import add_dep_helper

F = 128          # feature width (all layers)
P = 128          # partitions / block size
CORES = 8
BPC = 49         # blocks (positions) per core
NG = 64          # number of graphs
NCH = 7          # h-exchange chunks (position ranges)
CPP = BPC // NCH  # positions per chunk
VPAD = CORES * BPC * P
GP_BUFS = 168    # gather ring depth (layer-2 backlog cap)
BUDGET = 6       # gathers emitted per layer-1 block


def split_multi_waits(nc, max_waits: int = 1) -> int:
    n_split = 0
    f = nc.cur_f
    for bb in f.blocks:
        new_insts = []
        for inst in bb.instructions:
            si = inst.sync_info
            if si is not None and len(si.on_wait) > max_waits:
                waits = list(si.on_wait)
                extra, keep = waits[:-max_waits], waits[-max_waits:]
                for w in extra:
                    nop = mybir.InstNoOp(
                        name=nc.get_next_instruction_name(),
                        sync_info=mybir.SyncInfo(on_wait=[w], on_update=[]),
                        bass_nofuse=True,
                        engine=inst.engine,
                        ins=[],
                        outs=[],
                    )
                    nc.register_instruction(nop, overwrite=True)
                    new_insts.append(nop)
                inst.sync_info = mybir.SyncInfo(
                    on_wait=keep, on_update=list(si.on_update)
                )
                n_split += 1
            new_insts.append(inst)
        bb.instructions = new_insts
    return n_split


def _bf16(a):
    import ml_dtypes
    return np.asarray(a, dtype=np.float32).astype(ml_dtypes.bfloat16)


def _prep(x, edge_index, batch):
    """Host-side staging: node permutation, chunk-sorted tile structure,
    pre-gathered layer-1 stream, gather offsets, selection matrices."""
    import heapq

    n = x.shape[0]
    src = np.asarray(edge_index[0], dtype=np.int64)
    dst = np.asarray(edge_index[1], dtype=np.int64)
    w_reg = np.bincount(dst, minlength=n).astype(np.int64)
    deg = (w_reg + 1).astype(np.float64)  # incl self-loop (PyG GCNConv)

    nblocks = CORES * BPC
    order = np.argsort(-w_reg, kind="stable")
    heap = [(0, b) for b in range(nblocks)]
    heapq.heapify(heap)
    fill = np.zeros(nblocks, dtype=np.int64)
    node_block = np.empty(n, dtype=np.int64)
    node_slot = np.empty(n, dtype=np.int64)
    for nd in order:
        while True:
            load, b = heapq.heappop(heap)
            if fill[b] < P:
                break
        node_block[nd] = b
        node_slot[nd] = fill[b]
        fill[b] += 1
        if fill[b] < P:
            heapq.heappush(heap, (load + int(w_reg[nd]), b))

    # rank-match positions within each core (per-position max ~ mean)
    c_all = node_block // BPC
    ecnt = np.bincount(node_block[dst], minlength=nblocks).reshape(CORES, BPC)
    perm = np.empty(nblocks, dtype=np.int64)
    for c in range(CORES):
        order_c = np.argsort(-ecnt[c], kind="stable")
        for newp, old in enumerate(order_c):
            perm[c * BPC + old] = newp
    lb_all = perm[node_block]
    node_block = c_all * BPC + lb_all

    # chunk-major h_local row id (chunk j = positions [j*CPP, (j+1)*CPP))
    ch_all = lb_all // CPP
    rows_per_chunk = CORES * CPP * P
    pid2 = (ch_all * rows_per_chunk + c_all * (CPP * P)
            + (lb_all - ch_all * CPP) * P + node_slot)

    e_dst_b = node_block[dst]
    cnt2 = np.bincount(e_dst_b, minlength=nblocks).reshape(CORES, BPC)
    K = np.maximum(np.ceil(cnt2.max(axis=0) / P).astype(np.int64), 1)
    KMAX = int(K.max())
    T = int(K.sum())
    tile_base = np.concatenate([[0], np.cumsum(K)])[:-1]

    # per-edge placement: sort by (dst block, src chunk)
    e_src_ch = ch_all[src]
    eorder = np.lexsort((e_src_ch, e_dst_b))
    es_db = e_dst_b[eorder]
    es_srcp2 = pid2[src][eorder]
    es_srcnd = src[eorder]
    es_slot = node_slot[dst][eorder]
    es_norm = (1.0 / np.sqrt(deg[src] * deg[dst]))[eorder]
    es_ch = e_src_ch[eorder]
    bstart = np.concatenate([[0], np.cumsum(np.bincount(
        es_db, minlength=nblocks))])
    j_in = np.arange(len(es_db)) - bstart[es_db]
    tile_in = j_in // P
    part = j_in % P

    ecore = es_db // BPC
    elb = es_db % BPC
    gcol = tile_base[elb] + tile_in           # gather/tile column per core
    pcol = elb * KMAX + tile_in               # padded (uniform-stride) column

    # per-(core, tile) required chunk -> max over cores (SPMD uniform)
    req_ct = np.zeros((CORES, T), dtype=np.int64)
    np.maximum.at(req_ct, (ecore, gcol), es_ch)
    req = req_ct.max(axis=0)                  # [T]

    offs = np.zeros((CORES, P, T), dtype=np.int32)
    offs[ecore, part, gcol] = es_srcp2.astype(np.int32)

    sel = np.zeros((CORES, P, BPC * KMAX * P), dtype=np.float32)
    sel[ecore, part, pcol * P + es_slot] = es_norm
    sel = _bf16(sel)

    xf = _bf16(x)
    g1 = np.zeros((CORES, P, T * F), dtype=xf.dtype)
    g1v = g1.reshape(CORES, P, T, F)
    g1v[ecore, part, gcol] = xf[es_srcnd]

    # own rows in position-major order (layer-1 self source)
    x_own = np.zeros((CORES, BPC * P, F), dtype=xf.dtype)
    x_own[c_all, lb_all * P + node_slot] = xf

    bt = np.asarray(batch, dtype=np.int64)
    batchp = np.full((CORES, P, BPC), -1.0, dtype=np.float32)
    batchp[c_all, node_slot, lb_all] = bt.astype(np.float32)
    degself = np.ones((CORES, P, BPC), dtype=np.float32)
    degself[c_all, node_slot, lb_all] = deg.astype(np.float32)

    cnt = np.bincount(bt, minlength=NG).astype(np.float32)[:, None]
    return dict(offs=offs, sel=sel, g1=g1, x_own=x_own, batchp=batchp,
                degself=degself, cnt=cnt, K=K.tolist(), T=T, KMAX=KMAX,
                tile_base=tile_base.tolist(), req=req.tolist())


def _build(K, T, KMAX, tile_base, req):
    f32 = mybir.dt.float32
    bf16 = mybir.dt.bfloat16
    AF = mybir.ActivationFunctionType
    nc = bass.Bass(dynamic_dma_scratch_size=65536)

    g1_p = nc.declare_dram_parameter("g1", [P, T * F], bf16, isOutput=False)
    xown_p = nc.declare_dram_parameter("x_own", [BPC * P, F], bf16,
                                       isOutput=False)
    offs_p = nc.declare_dram_parameter("offs", [P, T], mybir.dt.int32,
                                       isOutput=False)
    sel_p = nc.declare_dram_parameter("sel", [P, BPC * KMAX * P], bf16,
                                      isOutput=False)
    batch_p = nc.declare_dram_parameter("batchp", [P, BPC], f32,
                                        isOutput=False)
    degself_p = nc.declare_dram_parameter("degself", [P, BPC], f32,
                                          isOutput=False)
    selfcol_p = nc.declare_dram_parameter("selfcol", [P, 1], f32,
                                          isOutput=False)
    iota_p = nc.declare_dram_parameter("iota", [P, P], f32, isOutput=False)
    w1_p = nc.declare_dram_parameter("W1", [F, F], bf16, isOutput=False)
    w2_p = nc.declare_dram_parameter("W2", [F, F], bf16, isOutput=False)
    wl_p = nc.declare_dram_parameter("Wl", [F, F], f32, isOutput=False)
    b1_p = nc.declare_dram_parameter("b1bc", [P, F], f32, isOutput=False)
    b2_p = nc.declare_dram_parameter("b2bc", [P, F], f32, isOutput=False)
    bl_p = nc.declare_dram_parameter("blbc", [NG, F], f32, isOutput=False)
    cnt_p = nc.declare_dram_parameter("cnt", [NG, 1], f32, isOutput=False)
    out_p = nc.declare_dram_parameter("out", [NG, F], f32, isOutput=True)

    rows_per_chunk = CORES * CPP * P

    # wave-major layer-2 unit list: (position, wave, tile-column)
    units = []
    for w in range(KMAX):
        for lb in range(BPC):
            if K[lb] > w:
                units.append((lb, w, tile_base[lb] + w))
    tiles_left = list(K)

    with tile.TileContext(nc) as tc:
        with (
            tc.tile_pool(name="dram", bufs=1, space="DRAM") as dram,
            tc.tile_pool(name="const", bufs=1) as cp,
            tc.tile_pool(name="g1p", bufs=3) as g1pool,
            tc.tile_pool(name="gp", bufs=GP_BUFS) as gp,
            tc.tile_pool(name="sel1p", bufs=3) as sel1p,
            tc.tile_pool(name="selwp", bufs=3) as selwp,
            tc.tile_pool(name="sp", bufs=4) as spool,
            tc.tile_pool(name="bp", bufs=4) as bpool,
            tc.tile_pool(name="ps", bufs=2, space="PSUM") as psp,
            tc.tile_pool(name="psagg", bufs=3, space="PSUM") as psagg,
            tc.tile_pool(name="psacc", bufs=1, space="PSUM") as psacc,
        ):
            ag_in = dram.tile([BPC * P, F], bf16)
            hc = [dram.tile([rows_per_chunk, F], bf16, addr_space="Shared",
                            name=f"hc{j}") for j in range(NCH)]
            h_local = dram.tile([VPAD, F], bf16)
            ar_in = dram.tile([F, NG], f32)
            ar_out = dram.tile([F, NG], f32, addr_space="Shared")

            offs_sb = cp.tile([P, T], mybir.dt.int32)
            nc.sync.dma_start(out=offs_sb[:], in_=offs_p[:])
            iota_sb = cp.tile([P, P], f32)
            nc.sync.dma_start(out=iota_sb[:], in_=iota_p[:])
            batch_sb = cp.tile([P, BPC], f32)
            nc.sync.dma_start(out=batch_sb[:], in_=batch_p[:])
            degself_sb = cp.tile([P, BPC], f32)
            nc.sync.dma_start(out=degself_sb[:], in_=degself_p[:])
            rdegself = cp.tile([P, BPC], f32)  # = dinv^2 per self-loop
            nc.vector.reciprocal(out=rdegself[:], in_=degself_sb[:])
            selfcol_sb = cp.tile([P, 1], f32)
            nc.sync.dma_start(out=selfcol_sb[:], in_=selfcol_p[:])
            ident = cp.tile([P, P], f32)
            nc.vector.tensor_tensor(
                out=ident[:],
                in0=selfcol_sb[:].to_broadcast([P, P]),
                in1=iota_sb[:],
                op=mybir.AluOpType.is_equal,
            )
            w1_sb = cp.tile([F, F], bf16)
            nc.sync.dma_start(out=w1_sb[:], in_=w1_p[:])
            w2_sb = cp.tile([F, F], bf16)
            nc.sync.dma_start(out=w2_sb[:], in_=w2_p[:])
            wl_sb = cp.tile([F, F], f32)
            nc.sync.dma_start(out=wl_sb[:], in_=wl_p[:])
            b1_sb = cp.tile([P, F], f32)
            nc.sync.dma_start(out=b1_sb[:], in_=b1_p[:])
            b2_sb = cp.tile([P, F], f32)
            nc.sync.dma_start(out=b2_sb[:], in_=b2_p[:])
            bl_sb = cp.tile([NG, F], f32)
            nc.sync.dma_start(out=bl_sb[:], in_=bl_p[:])
            cnt_sb = cp.tile([NG, 1], f32)
            nc.sync.dma_start(out=cnt_sb[:], in_=cnt_p[:])
            selfb = cp.tile([P, BPC * F], bf16)
            nc.sync.dma_start(
                out=selfb[:].rearrange("p (b f) -> p b f", f=F),
                in_=xown_p[:].rearrange("(b p) f -> p b f", p=P),
            )

            aggS = cp.tile([F, BPC * P], f32)       # layer-2 accumulators
            nc.vector.memset(aggS[:], 0.0)
            pool_acc = psacc.tile([F, NG], f32)

            state = {"uptr": 0, "gptr": 0, "landed": 0, "selw": None,
                     "selw_w": -1, "pool_n": 0, "copies": []}

            def epilogue2(lb):
                aggT = bpool.tile([F, P], bf16, tag="aggT")
                nc.vector.tensor_copy(out=aggT[:],
                                      in_=aggS[:, lb * P:(lb + 1) * P])
                psum_h = psp.tile([P, F], f32, tag="h")
                nc.tensor.matmul(out=psum_h[:], lhsT=aggT[:], rhs=w2_sb[:],
                                 start=True, stop=True)
                hb = bpool.tile([P, F], f32, tag="hb")
                nc.vector.tensor_add(out=hb[:], in0=psum_h[:], in1=b2_sb[:])
                hr = bpool.tile([P, F], bf16, tag="hr2")
                nc.scalar.activation(out=hr[:], in_=hb[:], func=AF.Relu)
                gb = bpool.tile([P, NG], bf16, tag="G")
                nc.vector.tensor_tensor(
                    out=gb[:],
                    in0=batch_sb[:, lb:lb + 1].to_broadcast([P, NG]),
                    in1=iota_sb[:, :NG],
                    op=mybir.AluOpType.is_equal,
                )
                state["pool_n"] += 1
                nc.tensor.matmul(out=pool_acc[:], lhsT=hr[:], rhs=gb[:],
                                 start=(state["pool_n"] == 1),
                                 stop=(state["pool_n"] == BPC))

            g_tiles = {}

            def emit_gather():
                lb, w, t = units[state["gptr"]]
                g = gp.tile([P, F], bf16, tag="g")
                reqrows = (req[t] + 1) * rows_per_chunk
                g_inst = nc.gpsimd.indirect_dma_start(
                    out=g[:],
                    out_offset=None,
                    in_=h_local[0:reqrows, :],
                    in_offset=bass.IndirectOffsetOnAxis(
                        ap=offs_sb[:, t:t + 1], axis=0),
                )
                # the indirect read of h_local is not range-tracked by the
                # tile dep machinery; tie it to the merge-copy it needs
                add_dep_helper(g_inst.ins, state["copies"][req[t]],
                               reason="gather waits h_local merge-copy")
                g_tiles[state["gptr"]] = g
                state["gptr"] += 1

            def emit_consume():
                lb, w, t = units[state["uptr"]]
                g = g_tiles.pop(state["uptr"])
                state["uptr"] += 1
                if w != state["selw_w"]:
                    selw = selwp.tile([P, BPC * P], bf16, tag="selw")
                    nc.scalar.dma_start(
                        out=selw[:].rearrange("p (b f) -> p b f", f=P),
                        in_=sel_p[:].rearrange(
                            "p (b k f) -> p b k f", k=KMAX, f=P)[:, :, w, :],
                    )
                    state["selw"] = selw
                    state["selw_w"] = w
                psum_t = psagg.tile([F, P], f32, tag="agg")
                nc.tensor.matmul(
                    out=psum_t[:], lhsT=g[:],
                    rhs=state["selw"][:, lb * P:(lb + 1) * P],
                    start=True, stop=True,
                )
                nc.vector.tensor_tensor(
                    out=aggS[:, lb * P:(lb + 1) * P],
                    in0=aggS[:, lb * P:(lb + 1) * P],
                    in1=psum_t[:],
                    op=mybir.AluOpType.add,
                )
                tiles_left[lb] -= 1
                if tiles_left[lb] == 0:
                    epilogue2(lb)

            # ---- layer 1 (+ interleaved layer-2 units) ----
            for lb in range(BPC):
                kb = K[lb]
                t0 = tile_base[lb]
                gt = g1pool.tile([P, KMAX * F], bf16, tag="g1")
                nc.sync.dma_start(out=gt[:, :kb * F],
                                  in_=g1_p[:, t0 * F:(t0 + kb) * F])
                selt = sel1p.tile([P, KMAX * P], bf16, tag="sel1")
                nc.sync.dma_start(
                    out=selt[:, :kb * P],
                    in_=sel_p[:, lb * KMAX * P:(lb * KMAX + kb) * P])
                psum_agg = psagg.tile([F, P], f32, tag="agg")
                sel_s = spool.tile([P, P], bf16, tag="sels")
                nc.scalar.activation(
                    out=sel_s[:], in_=ident[:], func=AF.Copy,
                    scale=rdegself[:, lb:lb + 1],
                )
                nc.tensor.matmul(
                    out=psum_agg[:], lhsT=selfb[:, lb * F:(lb + 1) * F],
                    rhs=sel_s[:], start=True, stop=False,
                )
                for t in range(kb):
                    nc.tensor.matmul(
                        out=psum_agg[:],
                        lhsT=gt[:, t * F:(t + 1) * F],
                        rhs=selt[:, t * P:(t + 1) * P],
                        start=False, stop=(t == kb - 1),
                    )
                aggT = bpool.tile([F, P], bf16, tag="aggT")
                nc.vector.tensor_copy(out=aggT[:], in_=psum_agg[:])
                psum_h = psp.tile([P, F], f32, tag="h")
                nc.tensor.matmul(out=psum_h[:], lhsT=aggT[:], rhs=w1_sb[:],
                                 start=True, stop=True)
                hb = bpool.tile([P, F], f32, tag="hb")
                nc.vector.tensor_add(out=hb[:], in0=psum_h[:], in1=b1_sb[:])
                hr = bpool.tile([P, F], bf16, tag="hr")
                nc.scalar.activation(out=hr[:], in_=hb[:], func=AF.Relu)
                nc.sync.dma_start(out=ag_in[lb * P:(lb + 1) * P, :],
                                  in_=hr[:])

                # layer-2 self-loop contribution (initializes aggS[:, lb])
                psum_s2 = psagg.tile([F, P], f32, tag="agg")
                nc.tensor.matmul(out=psum_s2[:], lhsT=hr[:], rhs=sel_s[:],
                                 start=True, stop=True)
                nc.vector.tensor_tensor(
                    out=aggS[:, lb * P:(lb + 1) * P],
                    in0=aggS[:, lb * P:(lb + 1) * P],
                    in1=psum_s2[:],
                    op=mybir.AluOpType.add,
                )

                if (lb + 1) % CPP == 0:
                    j = (lb + 1) // CPP - 1
                    nc.gpsimd.collective_compute(
                        "AllGather",
                        mybir.AluOpType.bypass,
                        replica_groups=[list(range(CORES))],
                        ins=[ag_in[j * CPP * P:(j + 1) * CPP * P, :]],
                        outs=[hc[j][:]],
                    )
                    c_inst = nc.scalar.dma_start(
                        out=h_local[j * rows_per_chunk:
                                    (j + 1) * rows_per_chunk, :],
                        in_=hc[j][:])
                    if state["copies"]:
                        # chain copies so copy_j's completion implies all
                        # earlier chunks have landed too
                        add_dep_helper(c_inst.ins, state["copies"][-1],
                                       reason="chain h_local merge-copies")
                    state["copies"].append(c_inst.ins)
                    state["landed"] += 1

                # gather-only during layer 1: decouple gpsimd from the
                # in-order PE queue. Pace emission (BUDGET per block) so AG
                # triggers on the in-order gpsimd queue are never stuck
                # behind a deep gather backlog; ring depth caps the total.
                budget = BUDGET
                while (budget > 0
                       and state["gptr"] < min(len(units), GP_BUFS)
                       and req[units[state["gptr"]][2]] < state["landed"]):
                    emit_gather()
                    budget -= 1

            # ---- rest of layer 2 ----
            while state["uptr"] < len(units):
                if state["gptr"] < len(units):
                    emit_gather()
                emit_consume()

            poolT_sb = cp.tile([F, NG], f32)
            nc.vector.tensor_copy(out=poolT_sb[:], in_=pool_acc[:])
            nc.sync.dma_start(out=ar_in[:], in_=poolT_sb[:])
            nc.gpsimd.collective_compute(
                "AllReduce",
                mybir.AluOpType.add,
                replica_groups=[list(range(CORES))],
                ins=[ar_in.opt()],
                outs=[ar_out.opt()],
            )
            poolT_ar = cp.tile([F, NG], f32)
            nc.sync.dma_start(out=poolT_ar[:], in_=ar_out[:])

            # head: out[g, :] = (sums[g] / max(cnt,1)) @ Wl + bl
            psum_o = psp.tile([NG, F], f32, tag="o")
            nc.tensor.matmul(out=psum_o[:], lhsT=poolT_ar[:], rhs=wl_sb[:],
                             start=True, stop=True)
            cmax = cp.tile([NG, 1], f32)
            nc.vector.tensor_scalar(out=cmax[:], in0=cnt_sb[:], scalar1=1.0,
                                    scalar2=None, op0=mybir.AluOpType.max)
            rcnt = cp.tile([NG, 1], f32)
            nc.vector.reciprocal(out=rcnt[:], in_=cmax[:])
            osc = cp.tile([NG, F], f32)
            nc.scalar.activation(out=osc[:], in_=psum_o[:], func=AF.Copy,
                                 scale=rcnt[:])
            ofin = cp.tile([NG, F], f32)
            nc.vector.tensor_add(out=ofin[:], in0=osc[:], in1=bl_sb[:])
            nc.sync.dma_start(out=out_p[:], in_=ofin[:])

    split_multi_waits(nc)
    return nc


def _run(inputs, trace=False):
    x = np.asarray(inputs["x"], dtype=np.float32)
    pp = _prep(x, np.asarray(inputs["edge_index"]),
               np.asarray(inputs["batch"]))

    iota = np.tile(np.arange(P, dtype=np.float32), (P, 1))
    w1 = _bf16(inputs["W1"])
    w2 = _bf16(inputs["W2"])
    wl = np.asarray(inputs["Wl"], dtype=np.float32)
    b1bc = np.tile(np.asarray(inputs["b1"], dtype=np.float32), (P, 1))
    b2bc = np.tile(np.asarray(inputs["b2"], dtype=np.float32), (P, 1))
    blbc = np.tile(np.asarray(inputs["bl"], dtype=np.float32), (NG, 1))

    nc = _build(pp["K"], pp["T"], pp["KMAX"], pp["tile_base"], pp["req"])
    in_maps = []
    for c in range(CORES):
        in_maps.append({
            "g1": pp["g1"][c],
            "x_own": pp["x_own"][c],
            "offs": pp["offs"][c],
            "sel": pp["sel"][c],
            "batchp": pp["batchp"][c],
            "degself": pp["degself"][c],
            "selfcol": np.arange(P, dtype=np.float32)[:, None],
            "iota": iota,
            "cnt": pp["cnt"],
            "W1": w1, "W2": w2, "Wl": wl,
            "b1bc": b1bc, "b2bc": b2bc, "blbc": blbc,
        })
    res = run_bass_kernel_spmd(nc, in_maps, list(range(CORES)), trace=trace)
    return res.results[0]["out"], res.exec_time_ns


def kernel(**inputs) -> np.ndarray:
    out, _ = _run(inputs)
    return out
